# revision 4
# baseline (speedup 1.0000x reference)
"""Bass/Tile TRN2 kernel for the sparse-attention (pointer-generator style)
attention module.

Reference computation (B=32, L=2048, N=1024):
    s         = s_t_hat @ W_dec.T + b_dec                     [B, N]
    attn_feat = tanh(enc_feat + s[:, None, :] + cov[..., None] * W_c)
    e         = einsum('bln,n->bl', attn_feat, W_v)           [B, L]
    attn      = softmax(e, axis=1) * mask; attn /= attn.sum(1) + 1e-12
    context   = einsum('bl,bln->bn', attn, h)                 [B, N]
    cov_new   = cov + attn

Strategy: pure data-parallel over batch (4 batches per core, 8 cores), no
collectives.  Memory-bound: each core streams its 32 MiB of enc_feat and
32 MiB of h through SBUF in 1 MiB DMAs.
"""

import numpy as np

import concourse.bacc as bacc
import concourse.bass as bass
import concourse.mybir as mybir
import concourse.tile as tile
from concourse.bass_utils import run_bass_kernel_spmd

B, L, N = 32, 2048, 1024
M = 8            # cores
BL = B // M      # local batches per core (4)
P = 128          # SBUF partitions
NCH = L // P     # L-chunks per batch (16)
Q = 2            # L-chunks per streaming DMA (1 MiB)
NT = NCH // Q    # streaming DMAs per batch per pass (8)
F32 = mybir.dt.float32
AF = mybir.ActivationFunctionType
ALU = mybir.AluOpType

_CACHED_NC = None


def _build_nc():
    nc = bacc.Bacc()

    h_d = nc.declare_dram_parameter("h", [BL, L, N], F32, isOutput=False)
    enc_d = nc.declare_dram_parameter("enc", [BL, L, N], F32, isOutput=False)
    covr_d = nc.declare_dram_parameter("cov_rows", [BL, L], F32, isOutput=False)
    covs_d = nc.declare_dram_parameter("cov_swz", [P, BL * NCH], F32, isOutput=False)
    mask_d = nc.declare_dram_parameter("mask_swz", [P, BL * NCH], F32, isOutput=False)
    stT_d = nc.declare_dram_parameter("stT", [N, BL], F32, isOutput=False)
    wdecT_d = nc.declare_dram_parameter("wdecT", [N, N], F32, isOutput=False)
    bdec_d = nc.declare_dram_parameter("b_dec", [1, N], F32, isOutput=False)
    wc_d = nc.declare_dram_parameter("w_c", [1, N], F32, isOutput=False)
    wv_d = nc.declare_dram_parameter("w_v", [1, N], F32, isOutput=False)

    attn_o = nc.declare_dram_parameter("attn_swz", [P, BL * NCH], F32, isOutput=True)
    ctx_o = nc.declare_dram_parameter("ctx", [BL, N], F32, isOutput=True)
    covn_o = nc.declare_dram_parameter("covnew_swz", [P, BL * NCH], F32, isOutput=True)

    # Stream views: chunk t covers L rows [t*Q*P, (t+1)*Q*P); partition p of the
    # tile holds rows t*Q*P + q*P + p for q in range(Q).
    enc_r = enc_d[:].rearrange("b (t q p) n -> b t p q n", q=Q, p=P)
    h_r = h_d[:].rearrange("b (t q p) n -> b t p q n", q=Q, p=P)

    with tile.TileContext(nc) as tc:
        with (
            tc.tile_pool(name="singles", bufs=1) as singles,
            tc.tile_pool(name="wdec_pool", bufs=2) as wdec_pool,
            tc.tile_pool(name="stream", bufs=4) as stream,
            tc.tile_pool(name="sums", bufs=3) as sums,
            tc.tile_pool(name="tanhs", bufs=3) as tanhs,
            tc.tile_pool(name="smalls", bufs=4) as smalls,
            tc.tile_pool(name="ctxs", bufs=2) as ctxs,
            tc.tile_pool(name="ps_big", bufs=2, space="PSUM") as ps_big,
            tc.tile_pool(name="ps_terms", bufs=2, space="PSUM") as ps_terms,
        ):
            # ---------------- setup ----------------
            # NOTE: all DMAs go through HWDGE (nc.sync) — SWDGE (gpsimd)
            # descriptor generation deadlocks against concurrent DVE activity
            # on this part (hardware port-sharing hazard).
            wv_b = singles.tile([P, N], F32)
            nc.sync.dma_start(
                out=wv_b[:],
                in_=bass.AP(tensor=wv_d, offset=0, ap=[[0, P], [1, N]]),
            )
            bdec_b = singles.tile([BL, N], F32)
            nc.sync.dma_start(
                out=bdec_b[:],
                in_=bass.AP(tensor=bdec_d, offset=0, ap=[[0, BL], [1, N]]),
            )
            ones2 = singles.tile([P, P], F32)
            nc.vector.memset(ones2[:], 1.0)

            # lhsT for the rank-2 "terms" matmul: partition 0 = ones,
            # partition 1 = coverage rows (original L order).
            lhsT_cov = singles.tile([2, BL, L], F32)
            nc.vector.memset(lhsT_cov[0:1, :, :], 1.0)
            for b in range(BL):
                nc.sync.dma_start(out=lhsT_cov[1:2, b, :], in_=covr_d[b : b + 1, :])

            # rhs for the terms matmul: partition 0 = s (per batch),
            # partition 1 = W_c.
            rhs_sw = singles.tile([2, BL, N], F32)
            for b in range(BL):
                nc.sync.dma_start(out=rhs_sw[1:2, b, :], in_=wc_d[0:1, :])

            mask_t = singles.tile([P, BL * NCH], F32)
            nc.sync.dma_start(out=mask_t[:], in_=mask_d[:])
            covs_t = singles.tile([P, BL * NCH], F32)
            nc.sync.dma_start(out=covs_t[:], in_=covs_d[:])

            e_t = singles.tile([P, BL * NCH], F32)
            attn_t = singles.tile([P, BL * NCH], F32)
            covn_t = singles.tile([P, BL * NCH], F32)
            scr = singles.tile([P, N], F32)  # dead full-size output of ttr

            # ---------------- dec_proj: s = s_t_hat @ W_dec.T + b_dec -------
            s_ps = ps_big.tile([BL, N], F32, tag="big")
            for kb in range(N // P):
                stT_t = smalls.tile([P, BL], F32, tag="stT")
                nc.sync.dma_start(out=stT_t[:], in_=stT_d[kb * P : (kb + 1) * P, :])
                wdecT_t = wdec_pool.tile([P, N], F32)
                nc.sync.dma_start(
                    out=wdecT_t[:], in_=wdecT_d[kb * P : (kb + 1) * P, :]
                )
                for nh in range(2):
                    nc.tensor.matmul(
                        s_ps[:, nh * 512 : (nh + 1) * 512],
                        stT_t[:],
                        wdecT_t[:, nh * 512 : (nh + 1) * 512],
                        start=(kb == 0),
                        stop=(kb == N // P - 1),
                    )
            s_sb = singles.tile([BL, N], F32)
            nc.vector.tensor_add(s_sb[:], s_ps[:], bdec_b[:])
            for b in range(BL):
                # cross-partition move b -> 0 into the rhs tile
                nc.sync.dma_start(out=rhs_sw[0:1, b, :], in_=s_sb[b : b + 1, :])

            # ---------------- pass B: e over enc_feat ----------------
            for b in range(BL):
                for t in range(NT):
                    enc_t = stream.tile([P, Q, N], F32, tag="stream")
                    nc.sync.dma_start(out=enc_t[:], in_=enc_r[b, t])
                    for q in range(Q):
                        c = t * Q + q
                        terms = ps_terms.tile([P, N], F32, tag="terms")
                        for nh in range(2):
                            nc.tensor.matmul(
                                terms[:, nh * 512 : (nh + 1) * 512],
                                lhsT_cov[:, b, c * P : (c + 1) * P],
                                rhs_sw[:, b, nh * 512 : (nh + 1) * 512],
                                start=True,
                                stop=True,
                            )
                        su = sums.tile([P, N], F32, tag="su")
                        nc.vector.tensor_add(su[:], enc_t[:, q, :], terms[:])
                        th = tanhs.tile([P, N], F32, tag="th")
                        nc.scalar.activation(th[:], su[:], AF.Tanh)
                        nc.vector.scalar_tensor_tensor(
                            out=scr[:],
                            in0=th[:],
                            scalar=1.0,
                            in1=wv_b[:],
                            op0=ALU.mult,
                            op1=ALU.mult,
                            accum_out=e_t[:, b * NCH + c : b * NCH + c + 1],
                        )

                # ------- masked softmax + renorm + coverage update for b ----
                sl = slice(b * NCH, (b + 1) * NCH)
                pexp = smalls.tile([P, NCH], F32, tag="pexp")
                nc.scalar.activation(pexp[:], e_t[:, sl], AF.Exp)
                pm = smalls.tile([P, NCH], F32, tag="pm")
                partial = smalls.tile([P, 1], F32, tag="partial")
                nc.vector.scalar_tensor_tensor(
                    out=pm[:],
                    in0=pexp[:],
                    scalar=1.0,
                    in1=mask_t[:, sl],
                    op0=ALU.mult,
                    op1=ALU.mult,
                    accum_out=partial[:],
                )
                tot = ps_big.tile([P, 1], F32, tag="big")
                nc.tensor.matmul(tot[:], ones2[:], partial[:], start=True, stop=True)
                rtot = smalls.tile([P, 1], F32, tag="rtot")
                nc.vector.reciprocal(rtot[:], tot[:])
                nc.vector.tensor_scalar_mul(attn_t[:, sl], pm[:], rtot[:])
                nc.vector.tensor_add(covn_t[:, sl], covs_t[:, sl], attn_t[:, sl])

            nc.sync.dma_start(out=attn_o[:], in_=attn_t[:])
            nc.sync.dma_start(out=covn_o[:], in_=covn_t[:])

            # ---------------- pass C: context = attn @ h ----------------
            for b in range(BL):
                ctx_ps = ps_big.tile([1, N], F32, tag="big")
                for t in range(NT):
                    h_t = stream.tile([P, Q, N], F32, tag="stream")
                    nc.sync.dma_start(out=h_t[:], in_=h_r[b, t])
                    for q in range(Q):
                        c = t * Q + q
                        for nh in range(2):
                            nc.tensor.matmul(
                                ctx_ps[:, nh * 512 : (nh + 1) * 512],
                                attn_t[:, b * NCH + c : b * NCH + c + 1],
                                h_t[:, q, nh * 512 : (nh + 1) * 512],
                                start=(c == 0),
                                stop=(c == NCH - 1),
                            )
                ctx_sb = ctxs.tile([1, N], F32, tag="ctx")
                nc.scalar.copy(ctx_sb[:], ctx_ps[:])
                nc.sync.dma_start(out=ctx_o[b : b + 1, :], in_=ctx_sb[:])

    nc.finalize()
    return nc


def _swz(x):
    """[BL, L] -> [P, BL*NCH]: column b*NCH+c, partition p <- x[b, c*P+p]."""
    return np.ascontiguousarray(
        x.reshape(BL, NCH, P).transpose(2, 0, 1).reshape(P, BL * NCH)
    )


def _unswz(y):
    """inverse of _swz"""
    return np.ascontiguousarray(
        y.reshape(P, BL, NCH).transpose(1, 2, 0).reshape(BL, L)
    )


def build_in_maps(inputs):
    return _build_in_maps(**inputs)


def _build_in_maps(h, enc_feat, attn_mask, s_t_hat, coverage, W_dec, b_dec, W_c, W_v):
    h = np.asarray(h, np.float32)
    enc_feat = np.asarray(enc_feat, np.float32)
    attn_mask = np.asarray(attn_mask, np.float32)
    s_t_hat = np.asarray(s_t_hat, np.float32)
    coverage = np.asarray(coverage, np.float32)
    wdecT = np.ascontiguousarray(np.asarray(W_dec, np.float32).T)
    bdec = np.ascontiguousarray(np.asarray(b_dec, np.float32).reshape(1, N))
    wc = np.ascontiguousarray(np.asarray(W_c, np.float32).reshape(1, N))
    wv = np.ascontiguousarray(np.asarray(W_v, np.float32).reshape(1, N))

    in_maps = []
    for core in range(M):
        sl = slice(core * BL, (core + 1) * BL)
        in_maps.append(
            {
                "h": np.ascontiguousarray(h[sl]),
                "enc": np.ascontiguousarray(enc_feat[sl]),
                "cov_rows": np.ascontiguousarray(coverage[sl]),
                "cov_swz": _swz(coverage[sl]),
                "mask_swz": _swz(attn_mask[sl]),
                "stT": np.ascontiguousarray(s_t_hat[sl].T),
                "wdecT": wdecT,
                "b_dec": bdec,
                "w_c": wc,
                "w_v": wv,
            }
        )
    return in_maps


def kernel(**inputs):
    global _CACHED_NC
    in_maps = build_in_maps(inputs)
    if _CACHED_NC is None:
        _CACHED_NC = _build_nc()
    res = run_bass_kernel_spmd(_CACHED_NC, in_maps, list(range(M)))

    attn = np.empty((B, L), np.float32)
    ctx = np.empty((B, N), np.float32)
    covn = np.empty((B, L), np.float32)
    for core in range(M):
        r = res.results[core]
        sl = slice(core * BL, (core + 1) * BL)
        attn[sl] = _unswz(r["attn_swz"])
        ctx[sl] = r["ctx"]
        covn[sl] = _unswz(r["covnew_swz"])
    return attn, ctx, covn


# revision 5
# speedup vs baseline: 1.4391x; 1.4391x over previous
"""Bass/Tile TRN2 kernel for the sparse-attention (pointer-generator style)
attention module.

Reference computation (B=32, L=2048, N=1024):
    s         = s_t_hat @ W_dec.T + b_dec                     [B, N]
    attn_feat = tanh(enc_feat + s[:, None, :] + cov[..., None] * W_c)
    e         = einsum('bln,n->bl', attn_feat, W_v)           [B, L]
    attn      = softmax(e, axis=1) * mask; attn /= attn.sum(1) + 1e-12
    context   = einsum('bl,bln->bn', attn, h)                 [B, N]
    cov_new   = cov + attn

Strategy: pure data-parallel over batch (4 batches per core, 8 cores), no
collectives.  Memory-bound: each core streams its 32 MiB of enc_feat (f32)
and 32 MiB of h (as two bf16 streams: h_hi + h_lo residual) through SBUF.

Numerics: PE fp32 matmuls run ~8x slower per column than bf16 (4 cyc/col
and an extra HI pass), so every matmul here is bf16.  The terms matmul
(s + cov*W_c) only perturbs the tanh argument by ~1e-5 absolute, which the
softmax is insensitive to; context keeps fp32-level accuracy through the
hi/lo split: context = a_hi@h_hi + a_lo@h_hi + a_hi@h_lo (a = attn).
"""

import ml_dtypes
import numpy as np

import concourse.bacc as bacc
import concourse.bass as bass
import concourse.bass_isa as bass_isa
import concourse.mybir as mybir
import concourse.tile as tile
from concourse.bass_utils import run_bass_kernel_spmd

B, L, N = 32, 2048, 1024
M = 8            # cores
BL = B // M      # local batches per core (4)
P = 128          # SBUF partitions
NCH = L // P     # L-chunks per batch (16)
Q = 2            # L-chunks per streaming DMA
NT = NCH // Q    # streaming DMAs per batch per pass (8)
F32 = mybir.dt.float32
BF16 = mybir.dt.bfloat16
AF = mybir.ActivationFunctionType
ALU = mybir.AluOpType
NPBF16 = ml_dtypes.bfloat16

_CACHED_NC = None


def _build_nc():
    nc = bacc.Bacc()

    hh_d = nc.declare_dram_parameter("h_hi", [BL, L, N], BF16, isOutput=False)
    hl_d = nc.declare_dram_parameter("h_lo", [BL, L, N], BF16, isOutput=False)
    enc_d = nc.declare_dram_parameter("enc", [BL, L, N], F32, isOutput=False)
    covr_d = nc.declare_dram_parameter("cov_rows", [BL, L], BF16, isOutput=False)
    covs_d = nc.declare_dram_parameter("cov_swz", [P, BL * NCH], F32, isOutput=False)
    mask_d = nc.declare_dram_parameter("mask_swz", [P, BL * NCH], F32, isOutput=False)
    stT_d = nc.declare_dram_parameter("stT", [N, BL], BF16, isOutput=False)
    wdecT_d = nc.declare_dram_parameter("wdecT", [N, N], BF16, isOutput=False)
    bdec_d = nc.declare_dram_parameter("b_dec", [1, N], F32, isOutput=False)
    wc_d = nc.declare_dram_parameter("w_c", [1, N], BF16, isOutput=False)
    wv_d = nc.declare_dram_parameter("w_v", [1, N], F32, isOutput=False)

    attn_o = nc.declare_dram_parameter("attn_swz", [P, BL * NCH], F32, isOutput=True)
    ctx_o = nc.declare_dram_parameter("ctx", [BL, N], F32, isOutput=True)
    covn_o = nc.declare_dram_parameter("covnew_swz", [P, BL * NCH], F32, isOutput=True)

    # Stream views: chunk t covers L rows [t*Q*P, (t+1)*Q*P); partition p of
    # the tile holds rows t*Q*P + q*P + p for q in range(Q).
    enc_r = enc_d[:].rearrange("b (t q p) n -> b t p q n", q=Q, p=P)
    hh_r = hh_d[:].rearrange("b (t q p) n -> b t p q n", q=Q, p=P)
    hl_r = hl_d[:].rearrange("b (t q p) n -> b t p q n", q=Q, p=P)

    with tile.TileContext(nc) as tc:
        with (
            tc.tile_pool(name="singles", bufs=1) as singles,
            tc.tile_pool(name="wdec_pool", bufs=2) as wdec_pool,
            tc.tile_pool(name="stream", bufs=4) as stream,
            tc.tile_pool(name="hstream", bufs=6) as hstream,
            tc.tile_pool(name="sums", bufs=3) as sums,
            tc.tile_pool(name="tanhs", bufs=3) as tanhs,
            tc.tile_pool(name="smalls", bufs=4) as smalls,
            tc.tile_pool(name="ctxs", bufs=2) as ctxs,
            tc.tile_pool(name="ps_big", bufs=2, space="PSUM") as ps_big,
            tc.tile_pool(name="ps_terms", bufs=2, space="PSUM") as ps_terms,
        ):
            # ---------------- setup ----------------
            # NOTE: all DMAs go through HWDGE (nc.sync) — SWDGE (gpsimd)
            # descriptor generation deadlocks against concurrent DVE activity
            # on this part (hardware port-sharing hazard).
            wv_b = singles.tile([P, N], F32)
            nc.sync.dma_start(
                out=wv_b[:],
                in_=bass.AP(tensor=wv_d, offset=0, ap=[[0, P], [1, N]]),
            )
            bdec_b = singles.tile([BL, N], F32)
            nc.sync.dma_start(
                out=bdec_b[:],
                in_=bass.AP(tensor=bdec_d, offset=0, ap=[[0, BL], [1, N]]),
            )

            # lhsT for the rank-2 "terms" matmul: partition 0 = ones,
            # partition 1 = coverage rows (original L order).  bf16.
            lhsT_cov = singles.tile([2, BL, L], BF16)
            nc.vector.memset(lhsT_cov[0:1, :, :], 1.0)
            for b in range(BL):
                nc.sync.dma_start(out=lhsT_cov[1:2, b, :], in_=covr_d[b : b + 1, :])

            # rhs for the terms matmul: partition 0 = s (per batch),
            # partition 1 = W_c.  bf16.
            rhs_sw = singles.tile([2, BL, N], BF16)
            for b in range(BL):
                nc.sync.dma_start(out=rhs_sw[1:2, b, :], in_=wc_d[0:1, :])

            mask_t = singles.tile([P, BL * NCH], F32)
            nc.sync.dma_start(out=mask_t[:], in_=mask_d[:])
            covs_t = singles.tile([P, BL * NCH], F32)
            nc.sync.dma_start(out=covs_t[:], in_=covs_d[:])

            e_t = singles.tile([P, BL * NCH], F32)
            attn_t = singles.tile([P, BL * NCH], F32)
            attn_hi = singles.tile([P, BL * NCH], BF16)
            attn_lo = singles.tile([P, BL * NCH], BF16)
            attn_lof = singles.tile([P, BL * NCH], F32)
            covn_t = singles.tile([P, BL * NCH], F32)
            scr = singles.tile([P, N], F32)  # dead full-size output of stt

            # ---------------- dec_proj: s = s_t_hat @ W_dec.T + b_dec -------
            s_ps = ps_big.tile([BL, N], F32, tag="big")
            for kb in range(N // P):
                stT_t = smalls.tile([P, BL], BF16, tag="stT")
                nc.sync.dma_start(out=stT_t[:], in_=stT_d[kb * P : (kb + 1) * P, :])
                wdecT_t = wdec_pool.tile([P, N], BF16)
                nc.sync.dma_start(
                    out=wdecT_t[:], in_=wdecT_d[kb * P : (kb + 1) * P, :]
                )
                for nh in range(2):
                    nc.tensor.matmul(
                        s_ps[:, nh * 512 : (nh + 1) * 512],
                        stT_t[:],
                        wdecT_t[:, nh * 512 : (nh + 1) * 512],
                        start=(kb == 0),
                        stop=(kb == N // P - 1),
                    )
            s_sb = singles.tile([BL, N], F32)
            nc.vector.tensor_add(s_sb[:], s_ps[:], bdec_b[:])
            s_bf = singles.tile([BL, N], BF16)
            nc.vector.tensor_copy(s_bf[:], s_sb[:])
            for b in range(BL):
                # cross-partition move b -> 0 into the rhs tile
                nc.sync.dma_start(out=rhs_sw[0:1, b, :], in_=s_bf[b : b + 1, :])

            # ---------------- pass B: e over enc_feat ----------------
            for b in range(BL):
                for t in range(NT):
                    enc_t = stream.tile([P, Q, N], F32, tag="stream")
                    nc.sync.dma_start(out=enc_t[:], in_=enc_r[b, t])
                    for q in range(Q):
                        c = t * Q + q
                        terms = ps_terms.tile([P, N], F32, tag="terms")
                        for nh in range(2):
                            nc.tensor.matmul(
                                terms[:, nh * 512 : (nh + 1) * 512],
                                lhsT_cov[:, b, c * P : (c + 1) * P],
                                rhs_sw[:, b, nh * 512 : (nh + 1) * 512],
                                start=True,
                                stop=True,
                            )
                        su = sums.tile([P, N], F32, tag="su")
                        nc.vector.tensor_add(su[:], enc_t[:, q, :], terms[:])
                        th = tanhs.tile([P, N], F32, tag="th")
                        nc.scalar.activation(th[:], su[:], AF.Tanh)
                        nc.vector.scalar_tensor_tensor(
                            out=scr[:],
                            in0=th[:],
                            scalar=1.0,
                            in1=wv_b[:],
                            op0=ALU.mult,
                            op1=ALU.mult,
                            accum_out=e_t[:, b * NCH + c : b * NCH + c + 1],
                        )

                # ------- masked softmax + renorm + coverage update for b ----
                sl = slice(b * NCH, (b + 1) * NCH)
                pexp = smalls.tile([P, NCH], F32, tag="pexp")
                nc.scalar.activation(pexp[:], e_t[:, sl], AF.Exp)
                pm = smalls.tile([P, NCH], F32, tag="pm")
                partial = smalls.tile([P, 1], F32, tag="partial")
                nc.vector.scalar_tensor_tensor(
                    out=pm[:],
                    in0=pexp[:],
                    scalar=1.0,
                    in1=mask_t[:, sl],
                    op0=ALU.mult,
                    op1=ALU.mult,
                    accum_out=partial[:],
                )
                tot = smalls.tile([P, 1], F32, tag="tot")
                nc.gpsimd.partition_all_reduce(
                    tot[:], partial[:], channels=P, reduce_op=bass_isa.ReduceOp.add
                )
                rtot = smalls.tile([P, 1], F32, tag="rtot")
                nc.vector.reciprocal(rtot[:], tot[:])
                nc.vector.tensor_scalar_mul(attn_t[:, sl], pm[:], rtot[:])
                nc.vector.tensor_add(covn_t[:, sl], covs_t[:, sl], attn_t[:, sl])
                # attn hi/lo bf16 split for the context matmul
                nc.vector.tensor_copy(attn_hi[:, sl], attn_t[:, sl])
                nc.vector.tensor_sub(attn_lof[:, sl], attn_t[:, sl], attn_hi[:, sl])
                nc.vector.tensor_copy(attn_lo[:, sl], attn_lof[:, sl])

            nc.sync.dma_start(out=attn_o[:], in_=attn_t[:])
            nc.sync.dma_start(out=covn_o[:], in_=covn_t[:])

            # ---------------- pass C: context = attn @ h ----------------
            for b in range(BL):
                ctx_ps = ps_big.tile([1, N], F32, tag="big")
                for t in range(NT):
                    hh_t = hstream.tile([P, Q, N], BF16, tag="hstream")
                    nc.sync.dma_start(out=hh_t[:], in_=hh_r[b, t])
                    hl_t = hstream.tile([P, Q, N], BF16, tag="hstream")
                    nc.sync.dma_start(out=hl_t[:], in_=hl_r[b, t])
                    for q in range(Q):
                        c = t * Q + q
                        col = slice(b * NCH + c, b * NCH + c + 1)
                        prods = (
                            (attn_hi, hh_t),
                            (attn_lo, hh_t),
                            (attn_hi, hl_t),
                        )
                        for pi, (a_til, h_til) in enumerate(prods):
                            for nh in range(2):
                                nc.tensor.matmul(
                                    ctx_ps[:, nh * 512 : (nh + 1) * 512],
                                    a_til[:, col],
                                    h_til[:, q, nh * 512 : (nh + 1) * 512],
                                    start=(c == 0 and pi == 0),
                                    stop=(c == NCH - 1 and pi == 2),
                                )
                ctx_sb = ctxs.tile([1, N], F32, tag="ctx")
                nc.scalar.copy(ctx_sb[:], ctx_ps[:])
                nc.sync.dma_start(out=ctx_o[b : b + 1, :], in_=ctx_sb[:])

    nc.finalize()
    return nc


def _swz(x):
    """[BL, L] -> [P, BL*NCH]: column b*NCH+c, partition p <- x[b, c*P+p]."""
    return np.ascontiguousarray(
        x.reshape(BL, NCH, P).transpose(2, 0, 1).reshape(P, BL * NCH)
    )


def _unswz(y):
    """inverse of _swz"""
    return np.ascontiguousarray(
        y.reshape(P, BL, NCH).transpose(1, 2, 0).reshape(BL, L)
    )


def build_in_maps(inputs):
    return _build_in_maps(**inputs)


def _build_in_maps(h, enc_feat, attn_mask, s_t_hat, coverage, W_dec, b_dec, W_c, W_v):
    h = np.asarray(h, np.float32)
    enc_feat = np.asarray(enc_feat, np.float32)
    attn_mask = np.asarray(attn_mask, np.float32)
    s_t_hat = np.asarray(s_t_hat, np.float32)
    coverage = np.asarray(coverage, np.float32)
    h_hi = h.astype(NPBF16)
    h_lo = (h - h_hi.astype(np.float32)).astype(NPBF16)
    wdecT = np.ascontiguousarray(np.asarray(W_dec, np.float32).T).astype(NPBF16)
    bdec = np.ascontiguousarray(np.asarray(b_dec, np.float32).reshape(1, N))
    wc = np.asarray(W_c, np.float32).reshape(1, N).astype(NPBF16)
    wv = np.ascontiguousarray(np.asarray(W_v, np.float32).reshape(1, N))

    in_maps = []
    for core in range(M):
        sl = slice(core * BL, (core + 1) * BL)
        in_maps.append(
            {
                "h_hi": np.ascontiguousarray(h_hi[sl]),
                "h_lo": np.ascontiguousarray(h_lo[sl]),
                "enc": np.ascontiguousarray(enc_feat[sl]),
                "cov_rows": np.ascontiguousarray(coverage[sl].astype(NPBF16)),
                "cov_swz": _swz(coverage[sl]),
                "mask_swz": _swz(attn_mask[sl]),
                "stT": np.ascontiguousarray(s_t_hat[sl].T.astype(NPBF16)),
                "wdecT": wdecT,
                "b_dec": bdec,
                "w_c": wc,
                "w_v": wv,
            }
        )
    return in_maps


def kernel(**inputs):
    global _CACHED_NC
    in_maps = build_in_maps(inputs)
    if _CACHED_NC is None:
        _CACHED_NC = _build_nc()
    res = run_bass_kernel_spmd(_CACHED_NC, in_maps, list(range(M)))

    attn = np.empty((B, L), np.float32)
    ctx = np.empty((B, N), np.float32)
    covn = np.empty((B, L), np.float32)
    for core in range(M):
        r = res.results[core]
        sl = slice(core * BL, (core + 1) * BL)
        attn[sl] = _unswz(r["attn_swz"])
        ctx[sl] = r["ctx"]
        covn[sl] = _unswz(r["covnew_swz"])
    return attn, ctx, covn
